# revision 7
# baseline (speedup 1.0000x reference)
"""Trainium2 Bass kernel for nn_Decoder (512-step LSTM scan, B=256, F=256).

Strategy: data-parallel over batch across 8 NeuronCores (32 batch/core).
After step 0 the LSTM input equals the hidden state, so W_ih+W_hh fold into
one combined weight for steps 1..511. Step 0 (and the initial_layer) runs on
host in numpy; each core runs 511 uniform recurrence steps.

Per-step device dataflow (batch-major [32, *] tiles):
  PE   : gates = hT.T @ wcT (+bias via ones-row matmul), fp32r, 6 MMs
  ACT  : sigma(i,f), tanh(g), sigma(o), tanh(c2)  (tanh(g) early: feeds DVE)
  DVE  : c2 = sig_f*c + sig_i*tanh_g ; h2 = sig_o*tanh(c2)
  DVE  : h2 -> hT via 4x 32x32 StreamTranspose + one f32r rounding copy
         (replaces PE transpose + ACT copy: fewer cross-engine sync hops,
          PE freed for the next step's matmuls)
"""
import sys

sys.path.insert(0, "/opt/trn_rl_repo")

import numpy as np

SEQ_LEN = 512
B, L, F = 256, 128, 256
NCORES = 8
BS = B // NCORES  # 32 batch per core

_CACHE = {}
VERSION = 4  # bump on every program change: forces a distinct NEFF cache key


def _sigmoid(x):
    out = np.empty_like(x)
    pos = x >= 0
    out[pos] = 1.0 / (1.0 + np.exp(-x[pos]))
    e = np.exp(x[~pos])
    out[~pos] = e / (1.0 + e)
    return out


def _build(steps):
    """Build + schedule the per-core Bass program (same program all cores)."""
    import concourse.mybir as mybir
    import concourse.tile as tile
    from concourse import bacc

    f32 = mybir.dt.float32
    f32r = mybir.dt.float32r
    AF = mybir.ActivationFunctionType

    nc = bacc.Bacc("TRN2", target_bir_lowering=False, debug=False)

    hT0_d = nc.dram_tensor("hT0", [F, BS], f32, kind="ExternalInput")
    c1_d = nc.dram_tensor("c1", [BS, F], f32, kind="ExternalInput")
    wcT_d = nc.dram_tensor("wcT", [F, 4 * F], f32, kind="ExternalInput")
    bias_d = nc.dram_tensor("bias", [1, 4 * F], f32, kind="ExternalInput")
    ones_d = nc.dram_tensor("ones", [1, BS], f32, kind="ExternalInput")
    # cache-buster: the neuron NEFF cache key ignores backend_config (the BIR),
    # so distinct programs with identical I/O shapes collide. Unique shape per
    # (VERSION, steps) forces a distinct HLO and cache entry.
    stag_d = nc.dram_tensor("stag", [VERSION, steps], f32, kind="ExternalInput")
    outs_d = nc.dram_tensor("outs", [SEQ_LEN, BS, F], f32, kind="ExternalOutput")

    with tile.TileContext(nc) as tc:
        with tc.tile_pool(name="const", bufs=1) as cpool, \
             tc.tile_pool(name="state", bufs=2) as spool, \
             tc.tile_pool(name="work", bufs=2) as wpool, \
             tc.tile_pool(name="h2p", bufs=6) as hpool, \
             tc.tile_pool(name="ps", bufs=2, space="PSUM") as psp:

            wc_sb = cpool.tile([128, 2 * 4 * F], f32r)
            nc.gpsimd.dma_start(
                out=wc_sb[:].rearrange("p (k n) -> p k n", k=2),
                in_=wcT_d.ap().rearrange("(k p) n -> p k n", p=128))
            bias_sb = cpool.tile([1, 4 * F], f32r)
            nc.gpsimd.dma_start(out=bias_sb[:], in_=bias_d.ap())
            ones_sb = cpool.tile([1, BS], f32r)
            nc.gpsimd.dma_start(out=ones_sb[:], in_=ones_d.ap())
            stag_sb = cpool.tile([1, 1], f32)
            nc.sync.dma_start(out=stag_sb[:], in_=stag_d.ap()[0:1, 0:1])

            hT_cur = spool.tile([128, 2 * BS], f32r, tag="hT")
            nc.gpsimd.dma_start(
                out=hT_cur[:].rearrange("p (k b) -> p k b", k=2),
                in_=hT0_d.ap().rearrange("(k p) b -> p k b", p=128))
            c_cur = spool.tile([BS, F], f32, tag="c")
            nc.sync.dma_start(out=c_cur[:], in_=c1_d.ap())

            for t in range(1, steps + 1):
                psA = psp.tile([BS, 512], f32, tag="psA")
                psB = psp.tile([BS, 512], f32, tag="psB")
                for ps, off in ((psA, 0), (psB, 512)):
                    for k in range(2):
                        nc.tensor.matmul(
                            ps[:], lhsT=hT_cur[:, BS * k:BS * (k + 1)],
                            rhs=wc_sb[:, 4 * F * k + off: 4 * F * k + off + 512],
                            start=(k == 0), stop=False)
                    nc.tensor.matmul(
                        ps[:], lhsT=ones_sb[:], rhs=bias_sb[:, off:off + 512],
                        start=False, stop=True)

                sA = wpool.tile([BS, 512], f32, tag="sA")
                nc.scalar.activation(sA[:], psA[:], AF.Sigmoid)
                tg = wpool.tile([BS, F], f32, tag="tg")
                nc.scalar.activation(tg[:], psB[:, F:2 * F], AF.Tanh)
                sO = wpool.tile([BS, F], f32, tag="sO")
                nc.scalar.activation(sO[:], psB[:, 0:F], AF.Sigmoid)

                t2 = wpool.tile([BS, F], f32, tag="t2")
                nc.vector.tensor_mul(t2[:], sA[:, F:2 * F], c_cur[:])
                t1 = wpool.tile([BS, F], f32, tag="t1")
                nc.vector.tensor_mul(t1[:], sA[:, 0:F], tg[:])
                c_new = spool.tile([BS, F], f32, tag="c")
                nc.vector.tensor_add(c_new[:], t1[:], t2[:])
                tc_t = wpool.tile([BS, F], f32, tag="tc")
                nc.scalar.activation(tc_t[:], c_new[:], AF.Tanh)
                h2 = hpool.tile([BS, F], f32, tag="h2")
                nc.vector.tensor_mul(h2[:], sO[:], tc_t[:])

                nc.sync.dma_start(out=outs_d.ap()[t], in_=h2[:])

                if t < steps:
                    hTt = wpool.tile([128, 2 * BS], f32, tag="hTt")
                    for j4 in range(4):
                        nc.vector.transpose(
                            hTt[32 * j4:32 * (j4 + 1), :].rearrange(
                                "p (k b) -> p k b", k=2),
                            h2[:].rearrange("b (k g x) -> g b k x",
                                            k=2, g=4)[j4])
                    hT_new = spool.tile([128, 2 * BS], f32r, tag="hT")
                    nc.vector.tensor_copy(hT_new[:], hTt[:])
                    hT_cur = hT_new
                c_cur = c_new

    nc.compile()
    return nc


def _get_nc(steps):
    if steps not in _CACHE:
        _CACHE[steps] = _build(steps)
    return _CACHE[steps]


def _host_prep(x, last_feat, Wi, bi, W_ih, W_hh, b_ih, b_hh):
    x = np.asarray(x, np.float32)
    last_feat = np.asarray(last_feat, np.float32)
    Wi = np.asarray(Wi, np.float32); bi = np.asarray(bi, np.float32)
    W_ih = np.asarray(W_ih, np.float32); W_hh = np.asarray(W_hh, np.float32)
    b_ih = np.asarray(b_ih, np.float32); b_hh = np.asarray(b_hh, np.float32)

    z = x[0] @ Wi.T + bi                       # [B, F]
    init = np.where(z > 0, z, np.expm1(z)).astype(np.float32)  # elu

    bsum = b_ih + b_hh
    g0 = last_feat @ W_ih.T + init @ W_hh.T + bsum   # [B, 4F] order i,f,g,o
    i0, f0, g0g, o0 = (g0[:, 0:F], g0[:, F:2*F], g0[:, 2*F:3*F], g0[:, 3*F:4*F])
    c1 = _sigmoid(f0) * init + _sigmoid(i0) * np.tanh(g0g)
    h1 = (_sigmoid(o0) * np.tanh(c1)).astype(np.float32)
    c1 = c1.astype(np.float32)

    # combined recurrent weight, rows reordered [i, f, o, g]
    Wc = W_ih + W_hh                            # [4F, F]
    perm = np.concatenate([np.arange(0, F), np.arange(F, 2*F),
                           np.arange(3*F, 4*F), np.arange(2*F, 3*F)])
    wcT = np.ascontiguousarray(Wc[perm].T)      # [F, 4F] gate order i,f,o,g
    bias_row = np.ascontiguousarray(bsum[perm][None, :])  # [1, 4F]
    return h1, c1, wcT, bias_row


_steps_of = [SEQ_LEN - 1]


def _in_maps(inputs, steps=None):
    _steps_of[0] = steps or _steps_of[0]
    h1, c1, wcT, bias_row = _host_prep(
        inputs["x"], inputs["last_feat"], inputs["Wi"], inputs["bi"],
        inputs["W_ih"], inputs["W_hh"], inputs["b_ih"], inputs["b_hh"])
    ones = np.ones((1, BS), np.float32)
    maps = []
    for ci in range(NCORES):
        s = slice(ci * BS, (ci + 1) * BS)
        maps.append(dict(
            hT0=np.ascontiguousarray(h1[s].T),
            c1=np.ascontiguousarray(c1[s]),
            wcT=wcT, bias=bias_row, ones=ones,
            stag=np.zeros((VERSION, _steps_of[0]), np.float32)))
    return maps


def kernel(x, last_feat, Wi, bi, W_ih, W_hh, b_ih, b_hh, Wo, bo,
           _steps=SEQ_LEN - 1):
    from concourse.bass_utils import run_bass_kernel_spmd

    h1, c1, wcT, bias_row = _host_prep(x, last_feat, Wi, bi, W_ih, W_hh,
                                       b_ih, b_hh)
    ones = np.ones((1, BS), np.float32)
    in_maps = []
    for ci in range(NCORES):
        s = slice(ci * BS, (ci + 1) * BS)
        in_maps.append(dict(
            hT0=np.ascontiguousarray(h1[s].T),
            c1=np.ascontiguousarray(c1[s]),
            wcT=wcT, bias=bias_row, ones=ones))

    for m in in_maps:
        m["stag"] = np.zeros((VERSION, _steps), np.float32)
    nc = _get_nc(_steps)
    res = run_bass_kernel_spmd(nc, in_maps, core_ids=list(range(NCORES)))

    outs = np.concatenate([r["outs"] for r in res.results], axis=1)  # [S, B, F]
    outs[0] = h1
    return np.ascontiguousarray(outs).reshape(B, SEQ_LEN, F)

